# revision 5
# baseline (speedup 1.0000x reference)
"""Multi-head attention (B=2,S=2048,D=1024,H=16) on 8 TRN2 NeuronCores.

Sharding: core c = b*4 + g handles batch b, head-group g (4 heads, 256 dims).
Per core (all matmuls bf16, fp32 PSUM accumulation, sT = scores transposed
[j-part, i-free] layout):
  A) projections:  qhT/khzT [256, 2048] (k pre-masked+pre-scaled on host),
     vh_aug [2048, 4*65] with a ones column per head (row-sum trick).
  B) per (head, j-block J): scores via matmul, causal-skip (i >= J*128),
     pad bias via ACT exp per-partition bias, causal diagonal via a
     pad-masked triangle add, exp -> unnormalized probs (bf16),
     PV accumulate into ctxT [65, 2048] PSUM (row 64 = row sums Z).
     Normalize via reciprocal + K=1 ones-matmul partition-broadcast.
  C) AllGather ctx over the 4 cores of each batch -> full ctx [1024, 2048].
  D) out-projection: this core's 256 output columns, outT [256, 2048].
Host: shards/transposes inputs, assembles output, and exactly fixes up
"fully-masked" rows (rows whose visible keys are all padded -> device
produces NaN there by construction) with a numpy reference computation.
"""
import sys
sys.path.insert(0, '/opt/trn_rl_repo')

import numpy as np
import ml_dtypes

import concourse.bass as bass
import concourse.mybir as mybir
from concourse.bass_utils import run_bass_kernel_spmd
from concourse.tile import TileContext

F32 = mybir.dt.float32
BF16 = mybir.dt.bfloat16
AF = mybir.ActivationFunctionType

B, S, D, H, HD = 2, 2048, 1024, 16, 64
G = 4          # head groups (cores per batch)
HPG = 4        # heads per group
DG = HPG * HD  # 256 dims per group
NB = S // 128  # 16 j/i blocks
NEG = -10000.0
CHUNK = 1024   # i-column chunk for scores/exp
SUB = 512      # matmul moving-dim sub-chunk

_cache = {}


def _split_multi_waits(nc):
    """This walrus build accepts at most ONE sync wait per instruction;
    split extras onto same-engine NoOps inserted just before (engine
    queues are in-order, so semantics are preserved)."""
    for fn in nc.m.functions:
        for bb in fn.blocks:
            insts = bb.instructions
            idx = 0
            while idx < len(insts):
                inst = insts[idx]
                si = inst.sync_info
                if si is not None and si.on_wait is not None and len(si.on_wait) > 1:
                    waits = list(si.on_wait)
                    for kk, w in enumerate(waits[:-1]):
                        nop = mybir.InstNoOp(
                            name=f"{inst.name}-ws{kk}", engine=inst.engine,
                            ins=[], outs=[])
                        nop.sync_info = mybir.SyncInfo(on_wait=[w], on_update=[])
                        nc.register_instruction(nop, overwrite=True)
                        insts.insert(idx, nop)
                        idx += 1
                    inst.sync_info = mybir.SyncInfo(
                        on_wait=[waits[-1]], on_update=list(si.on_update or []))
                idx += 1


def _build(causal: bool):
    nc = bass.Bass("TRN2", target_bir_lowering=False, debug=False, num_devices=8)

    qT = nc.declare_dram_parameter("qT", [D, S], BF16, isOutput=False)
    kTz = nc.declare_dram_parameter("kTz", [D, S], BF16, isOutput=False)
    vT = nc.declare_dram_parameter("vT", [D, S], BF16, isOutput=False)
    wqT = nc.declare_dram_parameter("wqT", [D, DG], BF16, isOutput=False)
    wkT = nc.declare_dram_parameter("wkT", [D, DG], BF16, isOutput=False)
    wvT = nc.declare_dram_parameter("wvT", [D, DG], BF16, isOutput=False)
    woT = nc.declare_dram_parameter("woT", [D, DG], BF16, isOutput=False)
    cpart = nc.declare_dram_parameter("cpart", [128, NB], F32, isOutput=False)
    padpart = nc.declare_dram_parameter("padpart", [128, NB], F32, isOutput=False)
    tri = nc.declare_dram_parameter("tri", [128, 128], F32, isOutput=False)
    outp = nc.declare_dram_parameter("outp", [DG, S], F32, isOutput=True)

    EB = D // 128  # 8 contraction blocks

    with TileContext(nc) as tc:
        with (
            tc.tile_pool(name="consts", bufs=1) as consts,
            tc.tile_pool(name="win", bufs=1) as win,
            tc.tile_pool(name="hold", bufs=1) as hold,
            tc.tile_pool(name="pt", bufs=3) as ptp,
            tc.tile_pool(name="dram", bufs=1, space="DRAM") as dram,
        ):
            # ---- constants ----
            t_c = consts.tile([128, NB], F32, tag="cpart")
            nc.sync.dma_start(out=t_c[:], in_=cpart[:])
            ones64 = consts.tile([1, 64], F32, tag="ones64")
            nc.vector.memset(ones64[:], 1.0)
            trz = []
            if causal:
                t_pad = consts.tile([128, NB], F32, tag="padpart")
                nc.sync.dma_start(out=t_pad[:], in_=padpart[:])
                t_tri = consts.tile([128, 128], F32, tag="tri")
                nc.sync.dma_start(out=t_tri[:], in_=tri[:])
                for J in range(NB):
                    t = consts.tile([128, 128], F32, tag=f"trz{J}")
                    nc.vector.tensor_scalar_mul(t[:], t_tri[:], t_pad[:, J:J + 1])
                    trz.append(t)

            # ---- stage A: projections ----
            stageA = (tc.tile_pool(name="xin", bufs=1),)
            xin = stageA[0].__enter__()
            psqk_cm = tc.tile_pool(name="psqk", bufs=1, space="PSUM")
            psA = psqk_cm.__enter__()
            # weights: 8 e-blocks each of [128, DG]
            wq_t, wk_t, wv_t = [], [], []
            for eb in range(EB):
                for (name, src, dst) in (("wq", wqT, wq_t), ("wk", wkT, wk_t),
                                         ("wv", wvT, wv_t)):
                    t = win.tile([128, DG], BF16, tag=f"{name}{eb}")
                    nc.sync.dma_start(out=t[:], in_=src[eb * 128:(eb + 1) * 128, :])
                    dst.append(t)
            # inputs resident: 8 e-blocks each of [128, S]
            q_t, k_t, v_t = [], [], []
            for eb in range(EB):
                for (name, src, dst) in (("q", qT, q_t), ("k", kTz, k_t),
                                         ("v", vT, v_t)):
                    t = xin.tile([128, S], BF16, tag=f"x{name}{eb}")
                    nc.sync.dma_start(out=t[:], in_=src[eb * 128:(eb + 1) * 128, :])
                    dst.append(t)

            # q/k projections -> qhT/khzT [2 x [128, S]] bf16
            qhT, khzT = [], []
            for (x_t, w_t, dst, nm) in ((q_t, wq_t, qhT, "qh"), (k_t, wk_t, khzT, "kh")):
                for db in range(2):
                    ps = psA.tile([128, S], F32, tag=f"psA{db}")
                    for eb in range(EB):
                        for s0 in range(0, S, SUB):
                            nc.tensor.matmul(
                                ps[:, s0:s0 + SUB],
                                w_t[eb][:, db * 128:(db + 1) * 128],
                                x_t[eb][:, s0:s0 + SUB],
                                start=(eb == 0), stop=(eb == EB - 1))
                    sb = hold.tile([128, S], BF16, tag=f"{nm}{db}")
                    nc.vector.tensor_copy(sb[:], ps[:])
                    dst.append(sb)

            psqk_cm.__exit__(None, None, None)
            psv_cm = tc.tile_pool(name="psvp", bufs=2, space="PSUM")
            psV = psv_cm.__enter__()

            # v projection -> vha[J] [128, HPG*65] bf16 (ones col per head)
            vha = []
            for J in range(NB):
                psv = psV.tile([128, DG], F32, tag="psv")
                for eb in range(EB):
                    nc.tensor.matmul(
                        psv[:], v_t[eb][:, J * 128:(J + 1) * 128], wv_t[eb][:],
                        start=(eb == 0), stop=(eb == EB - 1))
                t = hold.tile([128, HPG * 65], BF16, tag=f"vha{J}")
                # strided copy: head h dims -> cols [h*65, h*65+64)
                nc.vector.tensor_copy(
                    t[:].rearrange("p (h c) -> p h c", h=HPG)[:, :, 0:64],
                    psv[:].rearrange("p (h c) -> p h c", h=HPG))
                nc.vector.memset(
                    t[:].rearrange("p (h c) -> p h c", h=HPG)[:, :, 64:65], 1.0)
                vha.append(t)

            psv_cm.__exit__(None, None, None)
            stageA[0].__exit__(None, None, None)

            # ---- stages B: attention per head ----
            stageB = (tc.tile_pool(name="score", bufs=2, space="PSUM"),
                      tc.tile_pool(name="ctx", bufs=1, space="PSUM"))
            psS, psC = stageB[0].__enter__(), stageB[1].__enter__()
            ctx_part = dram.tile([DG, S], BF16)
            for h in range(HPG):
                db, r0 = h // 2, (h % 2) * 64
                kh_l = khzT[db]
                qh_l = qhT[db]
                ctxT = psC.tile([65, S], F32, tag="ctxT")
                for J in range(NB):
                    i0 = J * 128 if causal else 0
                    icols = S - i0
                    jsl = slice(J * 128, (J + 1) * 128)
                    nch = (icols + CHUNK - 1) // CHUNK
                    for ci in range(nch):
                        c0 = i0 + ci * CHUNK
                        cw = min(CHUNK, S - c0)
                        ps = psS.tile([128, CHUNK], F32, tag="score")
                        for s0 in range(0, cw, SUB):
                            sw = min(SUB, cw - s0)
                            nc.tensor.matmul(
                                ps[:, s0:s0 + sw],
                                kh_l[r0:r0 + 64, jsl],
                                qh_l[r0:r0 + 64, c0 + s0:c0 + s0 + sw],
                                start=True, stop=True)
                        if causal and ci == 0:
                            nc.vector.tensor_add(
                                ps[:, 0:128], ps[:, 0:128], trz[J][:])
                        pt = ptp.tile([128, CHUNK], BF16, tag="pt")
                        nc.scalar.activation(
                            pt[:, 0:cw], ps[:, 0:cw], AF.Exp,
                            bias=t_c[:, J:J + 1])
                        s = c0
                        while s < c0 + cw:
                            se = min((s // SUB + 1) * SUB, c0 + cw)
                            nc.tensor.matmul(
                                ctxT[:, s:se],
                                vha[J][:, h * 65:(h + 1) * 65],
                                pt[:, s - c0:se - c0],
                                start=(J == 0), stop=(J == NB - 1),
                                skip_group_check=True)
                            s = se
                # normalize: ctx/Z with Z = ctxT row 64
                ctxsb = hold.tile([65, S], F32, tag="ctxsb")
                nc.vector.tensor_copy(ctxsb[:], ctxT[:])
                z0 = hold.tile([1, S], F32, tag="z0")
                nc.sync.dma_start(out=z0[:], in_=ctxsb[64:65, :])
                nc.vector.reciprocal(z0[:], z0[:])
                recipb = psC.tile([64, S], F32, tag="ctxT")
                for s0 in range(0, S, SUB):
                    nc.tensor.matmul(
                        recipb[:, s0:s0 + SUB], ones64[:], z0[:, s0:s0 + SUB],
                        start=True, stop=True)
                ctxN = hold.tile([64, S], BF16, tag="ctxN")
                nc.vector.tensor_mul(ctxN[:], ctxsb[0:64, :], recipb[:])
                nc.sync.dma_start(out=ctx_part[h * 64:(h + 1) * 64, :], in_=ctxN[:])

            # ---- stage C: AllGather ctx across the 4 cores of this batch ----
            # Rank-order concat -> row d = g*256 + h*64 + r matches Wo order.
            ctx_all = dram.tile([D, S], BF16)
            nc.gpsimd.collective_compute(
                "AllGather", mybir.AluOpType.bypass,
                replica_groups=[[0, 1, 2, 3], [4, 5, 6, 7]],
                ins=[ctx_part.opt()], outs=[ctx_all.opt()])

            stageB[1].__exit__(None, None, None)
            stageB[0].__exit__(None, None, None)

            # ---- stage D: out projection (this core's 256 output cols) ----
            stageD = (tc.tile_pool(name="cxp", bufs=1),
                      tc.tile_pool(name="psD", bufs=1, space="PSUM"))
            cxp, psD = stageD[0].__enter__(), stageD[1].__enter__()
            # woT blocks: lhsT [d-blk 128, e-blk 128]; rhs ctx rows d.
            # gathered row for head dim d (global): rank gg, head hh, row rr:
            # ctx_all[gg*260 + hh*65 + rr] = ctx[d = gg*256 + hh*64 + rr]
            wo_t = []
            for dbk in range(EB):
                t = win.tile([128, DG], BF16, tag=f"wo{dbk}")
                nc.sync.dma_start(out=t[:], in_=woT[dbk * 128:(dbk + 1) * 128, :])
                wo_t.append(t)
            for ebk in range(2):
                pso = psD.tile([128, S], F32, tag=f"pso{ebk}")
                for dbk in range(EB):
                    cx = cxp.tile([128, S], BF16, tag=f"cx{dbk % 2}")
                    nc.sync.dma_start(
                        out=cx[:], in_=ctx_all[dbk * 128:(dbk + 1) * 128, :])
                    for s0 in range(0, S, SUB):
                        nc.tensor.matmul(
                            pso[:, s0:s0 + SUB],
                            wo_t[dbk][:, ebk * 128:(ebk + 1) * 128],
                            cx[:, s0:s0 + SUB],
                            start=(dbk == 0), stop=(dbk == EB - 1))
                osb = hold.tile([128, S], F32, tag=f"osb{ebk}")
                nc.vector.tensor_copy(osb[:], pso[:])
                nc.sync.dma_start(out=outp[ebk * 128:(ebk + 1) * 128, :], in_=osb[:])
            stageD[1].__exit__(None, None, None)
            stageD[0].__exit__(None, None, None)

    _split_multi_waits(nc)
    return nc


def _host_fixup_rows(out, q, k, v, attn_mask, Wq, Wk, Wv, Wo, causal, fm_rows, b):
    """Exact numpy recompute of reference for the given rows of batch b."""
    qf = np.asarray(q[b], np.float32)
    kf = np.asarray(k[b], np.float32)
    vf = np.asarray(v[b], np.float32)
    kh = (kf @ np.asarray(Wk, np.float32).T).reshape(S, H, HD)
    vh = (vf @ np.asarray(Wv, np.float32).T).reshape(S, H, HD)
    pad = np.asarray(attn_mask[b])
    scale = np.float32(1.0 / np.sqrt(HD))
    for i in fm_rows:
        qh = (qf[i] @ np.asarray(Wq, np.float32).T).reshape(H, HD)
        scores = np.einsum("hd,shd->hs", qh, kh).astype(np.float32) * scale
        if causal:
            scores = scores + np.where(np.arange(S)[None, :] > i,
                                       np.float32(NEG), np.float32(0.0))
        scores = np.where(pad[None, :] == 0, np.float32(NEG), scores)
        m = scores.max(axis=1, keepdims=True)
        p = np.exp(scores - m)
        p = p / p.sum(axis=1, keepdims=True)
        ctx = np.einsum("hs,shd->hd", p.astype(np.float32), vh).reshape(D)
        out[b, i, :] = ctx @ np.asarray(Wo, np.float32).T


def kernel(q, k, v, attn_mask, Wq, Wk, Wv, Wo, mask_future):
    out, _ = _run(q, k, v, attn_mask, Wq, Wk, Wv, Wo, mask_future)
    return out


def _run(q, k, v, attn_mask, Wq, Wk, Wv, Wo, mask_future, trace=False):
    causal = bool(int(mask_future))
    if causal not in _cache:
        _cache[causal] = _build(causal)
    nc = _cache[causal]

    bf = ml_dtypes.bfloat16
    q = np.asarray(q, np.float32)
    k = np.asarray(k, np.float32)
    v = np.asarray(v, np.float32)
    attn_mask = np.asarray(attn_mask)
    Wqf = np.asarray(Wq, np.float32)
    Wkf = np.asarray(Wk, np.float32)
    Wvf = np.asarray(Wv, np.float32)
    Wof = np.asarray(Wo, np.float32)

    scale = np.float32(1.0 / np.sqrt(HD))
    tri_np = np.where(np.arange(128)[:, None] > np.arange(128)[None, :],
                      np.float32(NEG), np.float32(0.0))
    in_maps = []
    for c in range(8):
        b, g = c // G, c % G
        rows = slice(g * DG, (g + 1) * DG)
        pad = attn_mask[b].astype(np.float32)          # [S]
        kz = k[b] * pad[:, None]
        padp = pad.reshape(NB, 128).T.copy()           # [128, NB]
        cp = (NEG * (1.0 - padp)).astype(np.float32)
        in_maps.append({
            "qT": np.ascontiguousarray(q[b].T).astype(bf),
            "kTz": np.ascontiguousarray(kz.T).astype(bf),
            "vT": np.ascontiguousarray(v[b].T).astype(bf),
            "wqT": np.ascontiguousarray((Wqf[rows] * scale).T).astype(bf),
            "wkT": np.ascontiguousarray(Wkf[rows].T).astype(bf),
            "wvT": np.ascontiguousarray(Wvf[rows].T).astype(bf),
            "woT": np.ascontiguousarray(Wof[rows].T).astype(bf),
            "cpart": cp,
            "padpart": padp.astype(np.float32),
            "tri": tri_np,
        })

    res = run_bass_kernel_spmd(nc, in_maps, core_ids=list(range(8)),
                               trace=trace)

    out = np.empty((B, S, D), np.float32)
    for c in range(8):
        b, g = c // G, c % G
        out[b, :, g * DG:(g + 1) * DG] = res.results[c]["outp"].T

    # host fixup of fully-masked rows (device yields NaN there by design)
    for b in range(B):
        m = attn_mask[b] != 0
        if causal:
            fm = np.where(~(np.cumsum(m) > 0))[0]
        else:
            fm = np.arange(S) if not m.any() else np.array([], np.int64)
        if len(fm):
            _host_fixup_rows(out, q, k, v, attn_mask, Wqf, Wkf, Wvf, Wof,
                             causal, fm, b)
    return out, res
